# revision 6
# baseline (speedup 1.0000x reference)
"""MoE dispatch (DispatchSF) Trainium2 Bass kernel — expert-parallel over 8 cores.

Problem: N=4096 tokens, D=1024 d_model, E=8 experts. For each expert e:
pack tokens with hot_mask[:, e] > 0 (in original order) into the first
`count` of N output slots, scaled by score[:, e]; zero-pad the rest.
One expert per NeuronCore; each core sees the full token buffer.

Device algorithm per core (token i <-> SBUF position (p, f), i = p*32 + f):
  1. prefix-sum of the mask: tensor_tensor_scan along the free dim +
     PE matmul with strict-upper-triangular ones for cross-partition offsets.
  2. compaction: 32 per-column indirect-DMA scatters write (token_id,
     score_bits) pairs into a [2N, 2] i32 buffer at row `excl_prefix` for
     selected tokens and row `N + token` (dump zone) otherwise. Every
     descriptor is valid and unique - no OOB semantics involved.
  3. load the packed first-N rows back: idxT[p, t]/scaleT[p, t] for output
     slot p*32 + t (zero-filled tail).
  4. 32 x (indirect row-gather of x [128 rows x 4 KiB] -> DVE scale -> store).

The (token, score) pair buffer is itself an output (opair); the host takes
out_tags from its first column.
"""

import os
import sys

import numpy as np

N, D, E = 4096, 1024, 8
P = 128
F = N // P  # 32

_CACHE = {}


def _ensure_path():
    for p in ("/opt/trn_rl_repo",):
        if p not in sys.path:
            sys.path.insert(0, p)


def _emit(tc, nc, ins, outs):
    """Emit the per-core device program. ins/outs: dicts of DRAM APs."""
    import concourse.bass as bass
    import concourse.mybir as mybir
    from concourse.masks import make_upper_triangular

    f32 = mybir.dt.float32
    i32 = mybir.dt.int32
    AO = mybir.AluOpType

    x = ins["x"]          # [N, D] f32
    mcol = ins["mcol"]    # [N, 1] i32  (this expert's hot_mask column)
    scol = ins["scol"]    # [N, 1] f32  (this expert's score column)
    odata = outs["odata"]  # [N, D] f32
    opair = outs["opair"]  # [2N, 2] i32: rows 0..N-1 = (token, score_bits)
    ocnt = outs["ocnt"]    # [1, 1] i32

    with (
        tc.tile_pool(name="small", bufs=1) as sp,
        tc.tile_pool(name="psum", bufs=1, space="PSUM") as pp,
        tc.tile_pool(name="xg", bufs=6) as xgp,
        tc.tile_pool(name="xs", bufs=6) as xsp,
    ):
        # --- load mask / scores, build constants ---
        mI = sp.tile([P, F], i32)
        nc.sync.dma_start(mI[:], mcol.rearrange("(p f) one -> p (f one)", p=P))
        mF = sp.tile([P, F], f32)
        nc.vector.tensor_copy(mF[:], mI[:])
        sF = sp.tile([P, F], f32)
        nc.sync.dma_start(sF[:], scol.rearrange("(p f) one -> p (f one)", p=P))

        utri = sp.tile([P, P], f32)
        make_upper_triangular(nc, utri[:], val=1.0, diag=False)
        onesc = sp.tile([P, 1], f32)
        nc.vector.memset(onesc[:], 1.0)

        tokI = sp.tile([P, F], i32)
        nc.gpsimd.iota(tokI[:], pattern=[[1, F]], base=0, channel_multiplier=F)

        # --- prefix sum over token order ---
        incl = sp.tile([P, F], f32)
        nc.vector.tensor_tensor_scan(
            incl[:], mF[:], mF[:], initial=0.0, op0=AO.add, op1=AO.bypass,
        )
        offs_ps = pp.tile([P, 1], f32, space="PSUM")
        nc.tensor.matmul(offs_ps[:], lhsT=utri[:], rhs=incl[:, F - 1:F],
                         start=True, stop=True)
        offsS = sp.tile([P, 1], f32)
        nc.vector.tensor_copy(offsS[:], offs_ps[:])

        # total count -> [1,1] via ones-matmul on per-partition totals
        cnt_ps = pp.tile([1, 1], f32, space="PSUM")
        nc.tensor.matmul(cnt_ps[:], lhsT=onesc[:], rhs=incl[:, F - 1:F],
                         start=True, stop=True)
        cntI = sp.tile([1, 1], i32)
        nc.vector.tensor_copy(cntI[:], cnt_ps[:])
        nc.sync.dma_start(ocnt[:], cntI[:])

        # exclusive prefix = incl - m + partition offset
        exclF = sp.tile([P, F], f32)
        nc.vector.tensor_tensor(exclF[:], incl[:], mF[:], op=AO.subtract)
        nc.vector.tensor_scalar(
            exclF[:], exclF[:], scalar1=offsS[:, :1], scalar2=None, op0=AO.add,
        )
        exclI = sp.tile([P, F], i32)
        nc.vector.tensor_copy(exclI[:], exclF[:])

        # dest row = m ? excl : N + token   (all rows valid & unique in [0, 2N))
        tok4096 = sp.tile([P, F], i32)
        nc.vector.tensor_scalar(tok4096[:], tokI[:], scalar1=N, scalar2=None,
                                op0=AO.add)
        t1 = sp.tile([P, F], i32)
        nc.vector.tensor_tensor(t1[:], exclI[:], tok4096[:], op=AO.subtract)
        t2 = sp.tile([P, F], i32)
        nc.vector.tensor_tensor(t2[:], t1[:], mI[:], op=AO.mult)
        destI = sp.tile([P, F], i32)
        nc.vector.tensor_tensor(destI[:], t2[:], tok4096[:], op=AO.add)

        # interleaved (token, score_bits) pairs: pairI[p, 2f]=tok, [p,2f+1]=score
        pairI = sp.tile([P, 2 * F], i32)
        pair3 = pairI[:].rearrange("p (f two) -> p f two", two=2)
        nc.vector.tensor_copy(pair3[:, :, 0], tokI[:])
        nc.vector.tensor_copy(pair3[:, :, 1], sF[:].bitcast(i32))

        # pre-zero the packed zone (rows 0..N-1) so the tail reads as
        # (token 0, scale 0.0)
        zpair = sp.tile([P, 2 * F], i32)
        nc.vector.memset(zpair[:], 0)
        # [128, 64] view of the packed zone (rows 0..N-1): partition p holds
        # rows 32p..32p+31 as interleaved pairs
        opair_pf = opair[0:N, :].rearrange("(p f) two -> p (f two)", p=P)
        nc.sync.dma_start(opair_pf, zpair[:])

        # 32 per-column pair scatters (each: 128 partitions x 8 B)
        for f in range(F):
            nc.gpsimd.indirect_dma_start(
                out=opair,
                out_offset=bass.IndirectOffsetOnAxis(ap=destI[:, f:f + 1], axis=0),
                in_=pair3[:, f, :],
                in_offset=None,
            )

        # load packed pairs back; deinterleave
        idxS = sp.tile([P, 2 * F], i32)
        nc.sync.dma_start(idxS[:], opair_pf)
        idx3 = idxS[:].rearrange("p (f two) -> p f two", two=2)
        idxT = sp.tile([P, F], i32)
        nc.vector.tensor_copy(idxT[:], idx3[:, :, 0])
        scaleT = sp.tile([P, F], f32)
        nc.vector.tensor_copy(scaleT[:], idx3[:, :, 1].bitcast(f32))

        # --- main dispatch loop: tile t covers slots p*32 + t ---
        odata_t = odata.rearrange("(p t) d -> p t d", t=F)
        for t in range(F):
            xg = xgp.tile([P, D], f32, tag="xg")
            nc.gpsimd.indirect_dma_start(
                out=xg[:],
                out_offset=None,
                in_=x,
                in_offset=bass.IndirectOffsetOnAxis(ap=idxT[:, t:t + 1], axis=0),
            )
            xs = xsp.tile([P, D], f32, tag="xs")
            nc.vector.tensor_scalar(
                xs[:], xg[:], scalar1=scaleT[:, t:t + 1], scalar2=None,
                op0=AO.mult,
            )
            nc.sync.dma_start(odata_t[:, t, :], xs[:])


def _build():
    _ensure_path()
    import concourse.bacc as bacc
    import concourse.mybir as mybir
    import concourse.tile as tile

    f32 = mybir.dt.float32
    i32 = mybir.dt.int32

    nc = bacc.Bacc(
        "TRN2",
        target_bir_lowering=False,
        debug=False,
        enable_asserts=True,
        num_devices=E,
    )
    ins = {
        "x": nc.dram_tensor("x", [N, D], f32, kind="ExternalInput").ap(),
        "mcol": nc.dram_tensor("mcol", [N, 1], i32, kind="ExternalInput").ap(),
        "scol": nc.dram_tensor("scol", [N, 1], f32, kind="ExternalInput").ap(),
    }
    outs = {
        "odata": nc.dram_tensor("odata", [N, D], f32, kind="ExternalOutput").ap(),
        "opair": nc.dram_tensor("opair", [2 * N, 2], i32, kind="ExternalOutput").ap(),
        "ocnt": nc.dram_tensor("ocnt", [1, 1], i32, kind="ExternalOutput").ap(),
    }
    with tile.TileContext(nc) as tc:
        _emit(tc, nc, ins, outs)
    nc.compile()
    return nc


def _get_nc():
    if "nc" not in _CACHE:
        _CACHE["nc"] = _build()
    return _CACHE["nc"]


def _install_ntff_hook():
    """Provide antenv.axon_hooks if the image lacks it (enables trace=True)."""
    try:
        from antenv.axon_hooks import get_axon_ntff_profile_hook  # noqa: F401
        return
    except ImportError:
        pass
    try:
        import types

        import antenv
        from trn_agent_boot.trn_boot import _ntff_profile_via_ctypes

        hook = _ntff_profile_via_ctypes("/opt/axon/libaxon_pjrt.so")
        mod = types.ModuleType("antenv.axon_hooks")
        mod.get_axon_ntff_profile_hook = lambda: hook
        mod.set_axon_ntff_profile_hook = lambda h: None
        sys.modules["antenv.axon_hooks"] = mod
        antenv.axon_hooks = mod
    except Exception:
        pass


def kernel(x, score, hot_mask, tag):
    _ensure_path()
    _install_ntff_hook()
    from concourse.bass_utils import run_bass_kernel_spmd

    x = np.ascontiguousarray(np.asarray(x, dtype=np.float32))
    score = np.asarray(score, dtype=np.float32)
    hot_mask = np.asarray(hot_mask, dtype=np.int32)

    nc = _get_nc()
    in_maps = [
        {
            "x": x,
            "mcol": np.ascontiguousarray(hot_mask[:, e:e + 1]),
            "scol": np.ascontiguousarray(score[:, e:e + 1]),
        }
        for e in range(E)
    ]
    trace = bool(int(os.environ.get("KERNEL_TRACE", "0")))
    res = run_bass_kernel_spmd(nc, in_maps, core_ids=list(range(E)), trace=trace)
    _CACHE["last_results"] = res

    out_data = np.stack([res.results[e]["odata"] for e in range(E)])
    out_tags = np.stack([res.results[e]["opair"][:N, 0:1] for e in range(E)])
    counts = np.array([res.results[e]["ocnt"][0, 0] for e in range(E)],
                      dtype=np.int32)
    return out_data, out_tags, counts


# revision 14
# speedup vs baseline: 1.3034x; 1.3034x over previous
"""MoE dispatch (DispatchSF) Trainium2 Bass kernel — expert-parallel over 8 cores.

Problem: N=4096 tokens, D=1024 d_model, E=8 experts. For each expert e:
pack tokens with hot_mask[:, e] > 0 (in original order) into the first
`count` of N output slots, scaled by score[:, e]; zero-pad the rest.
One expert per NeuronCore; each core sees the full token buffer.

Device algorithm per core (column-major token layout: token i <-> SBUF
position (p, f) with i = f*128 + p):
  1. exclusive prefix-sum of the mask over token order:
     within-column via PE matmul with strict-upper-triangular ones,
     cross-column via a [1, 32] tensor_tensor_scan + K=1 broadcast matmul.
  2. compaction: ONE dma_scatter_add ucode op scatters (token_id,
     score_bits) pairs into a 256B-strided [2N, 64] i32 output buffer at
     row `excl_prefix` for selected tokens, row `N + token` (dump zone)
     otherwise. The buffer arrives zero-filled (donated output), so
     add == write and the packed zone's tail stays zero.
  3. load the packed first-N rows back: idxT[p, t]/scaleT[p, t] for output
     slot p*32 + t (zero idx / zero scale in the tail).
  4. 32 x (indirect row-gather of x [128 rows x 4 KiB] -> DVE scale -> store).

Host slices out_tags from opair[:, :N, 0] and builds counts from ocnt.
"""

import os
import sys

import numpy as np

N, D, E = 4096, 1024, 8
P = 128
F = N // P  # 32
PAIR_STRIDE = 64  # i32 elements per opair row (256 B, dma_scatter_add stride req)

_CACHE = {}


def _ensure_path():
    for p in ("/opt/trn_rl_repo",):
        if p not in sys.path:
            sys.path.insert(0, p)


def _emit(tc, nc, ins, outs):
    """Emit the per-core device program. ins/outs: dicts of DRAM APs."""
    import concourse.mybir as mybir
    from concourse.masks import make_upper_triangular

    f32 = mybir.dt.float32
    i32 = mybir.dt.int32
    i16 = mybir.dt.int16
    AO = mybir.AluOpType

    x = ins["x"]          # [N, D] f32
    mcol = ins["mcol"]    # [128, 32] i32, col-major: [p, f] = mask[f*128 + p]
    scol = ins["scol"]    # [128, 32] f32, col-major
    odata = outs["odata"]  # [N, D] f32
    opair = outs["opair"]  # [2N, 64] i32; row s cols 0:2 = (token, score_bits)
    ocnt = outs["ocnt"]    # [1, 1] i32

    with (
        tc.tile_pool(name="small", bufs=1) as sp,
        tc.tile_pool(name="psum", bufs=1, space="PSUM") as pp,
        tc.tile_pool(name="xg", bufs=8) as xgp,
        tc.tile_pool(name="xs", bufs=8) as xsp,
    ):
        # --- load mask / scores (column-major), constants ---
        mF = sp.tile([P, F], f32)
        mI = sp.tile([P, F], i32)
        nc.sync.dma_start(mI[:], mcol)
        nc.vector.tensor_copy(mF[:], mI[:])
        sF = sp.tile([P, F], f32)
        nc.sync.dma_start(sF[:], scol)

        utri = sp.tile([P, P], f32)
        make_upper_triangular(nc, utri[:], val=1.0, diag=False)
        onescol = sp.tile([P, 1], f32)
        nc.vector.memset(onescol[:], 1.0)
        onesrow = sp.tile([1, P], f32)
        nc.vector.memset(onesrow[:], 1.0)

        tokI = sp.tile([P, F], i32)  # token id i = f*128 + p
        nc.gpsimd.iota(tokI[:], pattern=[[P, F]], base=0, channel_multiplier=1)

        # --- exclusive prefix over token order ---
        # within-column exclusive prefix (over partitions)
        excl_ps = pp.tile([P, F], f32, space="PSUM")
        nc.tensor.matmul(excl_ps[:], lhsT=utri[:], rhs=mF[:], start=True, stop=True)
        # per-column totals [1, 32]
        colsum_ps = pp.tile([1, F], f32, space="PSUM")
        nc.tensor.matmul(colsum_ps[:], lhsT=onescol[:], rhs=mF[:],
                         start=True, stop=True)
        # inclusive scan of column totals, then make exclusive
        colsumS = sp.tile([1, F], f32)
        nc.vector.tensor_copy(colsumS[:], colsum_ps[:])
        colincl = sp.tile([1, F], f32)
        nc.vector.tensor_tensor_scan(
            colincl[:], colsumS[:], colsumS[:], initial=0.0,
            op0=AO.add, op1=AO.bypass,
        )
        colexcl = sp.tile([1, F], f32)
        nc.vector.tensor_tensor(colexcl[:], colincl[:], colsumS[:],
                                op=AO.subtract)
        # broadcast column offsets to all partitions (K=1 matmul)
        coloff_ps = pp.tile([P, F], f32, space="PSUM")
        nc.tensor.matmul(coloff_ps[:], lhsT=onesrow[:], rhs=colexcl[:],
                         start=True, stop=True)
        coloffS = sp.tile([P, F], f32)
        nc.vector.tensor_copy(coloffS[:], coloff_ps[:])
        exclF = sp.tile([P, F], f32)
        nc.vector.tensor_tensor(exclF[:], excl_ps[:], coloffS[:], op=AO.add)
        exclI = sp.tile([P, F], i32)
        nc.vector.tensor_copy(exclI[:], exclF[:])

        # total count -> ocnt
        cntI = sp.tile([1, 1], i32)
        nc.vector.tensor_copy(cntI[:], colincl[:, F - 1:F])
        nc.sync.dma_start(ocnt[:], cntI[:])

        # dest row = m ? excl : N + token   (all rows valid & unique in [0, 2N))
        tok4096 = sp.tile([P, F], i32)
        nc.vector.tensor_scalar(tok4096[:], tokI[:], scalar1=N, scalar2=None,
                                op0=AO.add)
        t1 = sp.tile([P, F], i32)
        nc.vector.tensor_tensor(t1[:], exclI[:], tok4096[:], op=AO.subtract)
        t2 = sp.tile([P, F], i32)
        nc.vector.tensor_tensor(t2[:], t1[:], mI[:], op=AO.mult)
        destI = sp.tile([P, F], i32)
        nc.vector.tensor_tensor(destI[:], t2[:], tok4096[:], op=AO.add)

        # wrap dest to dma idx layout: idx for token i sits at [i%16, i//16],
        # int16, replicated to all 128 partitions.
        # (p, f) -> (p%16, 8f + p//16): fold partition groups via PE with
        # identity-slice selectors (engines can't address partition base 16),
        # then replicate 16 -> 128 partitions.
        destF2 = sp.tile([P, F], f32)
        nc.vector.tensor_copy(destF2[:], destI[:])
        ident = sp.tile([P, P], f32)
        from concourse.masks import make_identity
        make_identity(nc, ident[:])
        fold_ps = pp.tile([16, 8 * F], f32, space="PSUM")
        for g in range(8):
            nc.tensor.matmul(fold_ps[:, F * g:F * (g + 1)],
                             lhsT=ident[:, 16 * g:16 * (g + 1)],
                             rhs=destF2[:], start=True, stop=True)
        idx16 = sp.tile([P, N // 16], i16)
        idx16_3d = idx16[:].rearrange("p (f g) -> p f g", g=8)
        for g in range(8):
            nc.vector.tensor_copy(idx16_3d[0:16, :, g],
                                  fold_ps[0:16, F * g:F * (g + 1)])
        nc.sync.dma_start(idx16[16:32, :], idx16[0:16, :])
        nc.vector.tensor_copy(idx16[32:64, :], idx16[0:32, :])
        nc.vector.tensor_copy(idx16[64:128, :], idx16[0:64, :])

        # pre-zero the packed zone (rows 0..N-1, all cols): one contiguous
        # 1 MiB write, so the packed tail reads back as (token 0, scale 0.0)
        # without relying on donated-zero output buffers.
        zpair = sp.tile([P, N * PAIR_STRIDE // P], i32)
        nc.vector.memset(zpair[:], 0)
        nc.sync.dma_start(
            opair[0:N, :].rearrange("(p t) c -> p (t c)", p=P), zpair[:],
        )

        # interleaved (token, score_bits) pairs, token i at [i%128, 2*(i//128)]
        pairI = sp.tile([P, 2 * F], i32)
        pair3 = pairI[:].rearrange("p (f two) -> p f two", two=2)
        nc.vector.tensor_copy(pair3[:, :, 0], tokI[:])
        nc.vector.tensor_copy(pair3[:, :, 1], sF[:].bitcast(i32))

        # ONE compaction scatter: opair[dest, 0:2] += (token, score_bits)
        nc.gpsimd.dma_scatter_add(
            out_ap=opair[:, 0:2],
            in_ap=pair3[:, :, :],
            idxs_ap=idx16[:],
            num_idxs=N,
            num_idxs_reg=N,
            elem_size=2,
            elem_step=PAIR_STRIDE,
        )

        # load packed pairs back; slot s = p*32 + t at partition p
        idxS = sp.tile([P, 2 * F], i32)
        idx3 = idxS[:].rearrange("p (t two) -> p t two", two=2)
        nc.sync.dma_start(
            idx3[:, :, :],
            opair[0:N, :].rearrange("(p t) c -> p t c", p=P)[:, :, 0:2],
        )
        idxT = sp.tile([P, F], i32)
        nc.vector.tensor_copy(idxT[:], idx3[:, :, 0])
        scaleT = sp.tile([P, F], f32)
        nc.vector.tensor_copy(scaleT[:], idx3[:, :, 1].bitcast(f32))

        # --- main dispatch loop: tile t covers slots p*32 + t ---
        import concourse.bass as bass

        odata_t = odata.rearrange("(p t) d -> p t d", t=F)
        for t in range(F):
            xg = xgp.tile([P, D], f32, tag="xg")
            nc.gpsimd.indirect_dma_start(
                out=xg[:],
                out_offset=None,
                in_=x,
                in_offset=bass.IndirectOffsetOnAxis(ap=idxT[:, t:t + 1], axis=0),
            )
            xs = xsp.tile([P, D], f32, tag="xs")
            nc.vector.tensor_scalar(
                xs[:], xg[:], scalar1=scaleT[:, t:t + 1], scalar2=None,
                op0=AO.mult,
            )
            nc.sync.dma_start(odata_t[:, t, :], xs[:])


def _build():
    _ensure_path()
    import concourse.bacc as bacc
    import concourse.mybir as mybir
    import concourse.tile as tile

    f32 = mybir.dt.float32
    i32 = mybir.dt.int32

    nc = bacc.Bacc(
        "TRN2",
        target_bir_lowering=False,
        debug=False,
        enable_asserts=True,
        num_devices=E,
    )
    ins = {
        "x": nc.dram_tensor("x", [N, D], f32, kind="ExternalInput").ap(),
        "mcol": nc.dram_tensor("mcol", [P, F], i32, kind="ExternalInput").ap(),
        "scol": nc.dram_tensor("scol", [P, F], f32, kind="ExternalInput").ap(),
    }
    outs = {
        "odata": nc.dram_tensor("odata", [N, D], f32, kind="ExternalOutput").ap(),
        "opair": nc.dram_tensor("opair", [2 * N, PAIR_STRIDE], i32,
                                kind="ExternalOutput").ap(),
        "ocnt": nc.dram_tensor("ocnt", [1, 1], i32, kind="ExternalOutput").ap(),
    }
    with tile.TileContext(nc) as tc:
        _emit(tc, nc, ins, outs)
    nc.compile()
    return nc


def _get_nc():
    if "nc" not in _CACHE:
        _CACHE["nc"] = _build()
    return _CACHE["nc"]


def _install_ntff_hook():
    """Provide antenv.axon_hooks if the image lacks it (enables trace=True)."""
    try:
        from antenv.axon_hooks import get_axon_ntff_profile_hook  # noqa: F401
        return
    except ImportError:
        pass
    try:
        import types

        import antenv
        from trn_agent_boot.trn_boot import _ntff_profile_via_ctypes

        hook = _ntff_profile_via_ctypes("/opt/axon/libaxon_pjrt.so")
        mod = types.ModuleType("antenv.axon_hooks")
        mod.get_axon_ntff_profile_hook = lambda: hook
        mod.set_axon_ntff_profile_hook = lambda h: None
        sys.modules["antenv.axon_hooks"] = mod
        antenv.axon_hooks = mod
    except Exception:
        pass


def kernel(x, score, hot_mask, tag):
    _ensure_path()
    _install_ntff_hook()
    from concourse.bass_utils import run_bass_kernel_spmd

    x = np.ascontiguousarray(np.asarray(x, dtype=np.float32))
    score = np.asarray(score, dtype=np.float32)
    hot_mask = np.asarray(hot_mask, dtype=np.int32)

    nc = _get_nc()
    # column-major [p, f] = value[f*128 + p]
    in_maps = [
        {
            "x": x,
            "mcol": np.ascontiguousarray(hot_mask[:, e].reshape(F, P).T),
            "scol": np.ascontiguousarray(score[:, e].reshape(F, P).T),
        }
        for e in range(E)
    ]
    trace = bool(int(os.environ.get("KERNEL_TRACE", "0")))
    res = run_bass_kernel_spmd(nc, in_maps, core_ids=list(range(E)), trace=trace)
    _CACHE["last_results"] = res

    out_data = np.stack([res.results[e]["odata"] for e in range(E)])
    out_tags = np.stack([res.results[e]["opair"][:N, 0:1] for e in range(E)])
    counts = np.array([res.results[e]["ocnt"][0, 0] for e in range(E)],
                      dtype=np.int32)
    return out_data, out_tags, counts
